# revision 12
# baseline (speedup 1.0000x reference)
"""Trainium2 Bass kernel for nn_ComputeTargets (retinanet-style target assignment).

Strategy:
  - Data-parallel over batch: each of the 8 NeuronCores processes one image
    (anchors replicated across cores).
  - Per core: for each 128-anchor tile, compute the [128a x 128g] IoU matrix
    with the exact fp32 op tree of the reference (min/max/sub/mul/add + one
    true divide), reduce to max + first-argmax (eq * descending-weight trick,
    exact first-match semantics), gather the assigned GT row via a one-hot
    matmul on the TensorEngine, then batched epilogue computes states,
    one-hot class targets and regression targets.
"""

import sys

sys.path.insert(0, "/opt/trn_rl_repo")

import numpy as np

import concourse.bass as bass
import concourse.bacc as bacc
import concourse.mybir as mybir
from concourse.tile import TileContext
from concourse.bass_utils import run_bass_kernel_spmd

F32 = mybir.dt.float32
OP = mybir.AluOpType
AF = mybir.ActivationFunctionType
AX = mybir.AxisListType

B, G, A, C = 8, 128, 131072, 80
P = 128               # partitions
T_FULL = A // P       # 1024 anchor tiles
GROUP = 64            # tiles per PSUM-assigned flush group (64*8 cols = 512 = 1 bank)
CLS_DMA = 8           # tiles per cls DMA batch

_CACHE = {}


def build_kernel(T=T_FULL):
    nc = bacc.Bacc("TRN2", target_bir_lowering=False, debug=False)
    AA = T * P  # anchors handled

    # ---- I/O ----
    def pin(name, shape):
        return nc.declare_dram_parameter(name, list(shape), F32, isOutput=False)

    bcast = pin("bcast", [P, 6 * G])      # bx1|by1|bx2|by2|areab|desc  blocks of G cols
    iota80 = pin("iota80", [P, C])
    ann_rhs = pin("ann_rhs", [P, 8])      # x1,y1,x2,y2,label,0,0,0  (per-GT rows)
    ident = pin("ident", [P, P])
    panch = pin("panch", [P, 7 * T])      # AX1|AY1|AX2|AY2|AREA|WA|HA blocks of T cols

    cls_out = nc.declare_dram_parameter("cls", [AA, C], F32, isOutput=True)
    reg_out = nc.declare_dram_parameter("reg", [AA, 4], F32, isOutput=True)
    st_out = nc.declare_dram_parameter("st", [AA], F32, isOutput=True)

    with TileContext(nc) as tc:
        with (
            tc.tile_pool(name="const", bufs=1) as const_pool,
            tc.tile_pool(name="big", bufs=1) as big_pool,
            tc.tile_pool(name="work", bufs=3) as work,
            tc.tile_pool(name="ohp", bufs=3) as ohp,
            tc.tile_pool(name="clsb", bufs=3) as clsb,
            tc.tile_pool(name="ptr", bufs=3, space="PSUM") as ptr,
            tc.tile_pool(name="pasg", bufs=2, space="PSUM") as pasg,
        ):
            # ---- load constants ----
            # Chunked loads + same-engine "touch" ops: per-instruction
            # sync-wait slots are scarce, and Tile waits are per-engine
            # observed ticks — a cheap DVE read of each chunk makes the DVE
            # observe that chunk's DMA semaphores so the real consumers
            # need no new waits.
            touch_scr = const_pool.tile([P, 64], F32)
            touch_list = []

            def load_chunked(dst_ap, src_ap, cols, chunk=256):
                for c0 in range(0, cols, chunk):
                    c1 = min(cols, c0 + chunk)
                    nc.sync.dma_start(out=dst_ap[:, c0:c1], in_=src_ap[:, c0:c1])
                    touch_list.append(dst_ap[:, c0:c0 + 1])

            bc = const_pool.tile([P, 6 * G], F32)
            load_chunked(bc, bcast, 6 * G)
            bx1 = bc[:, 0 * G:1 * G]
            by1 = bc[:, 1 * G:2 * G]
            bx2 = bc[:, 2 * G:3 * G]
            by2 = bc[:, 3 * G:4 * G]
            areab = bc[:, 4 * G:5 * G]
            desc = bc[:, 5 * G:6 * G]

            io80 = const_pool.tile([P, C], F32)
            nc.sync.dma_start(out=io80[:], in_=iota80[:, :])
            annr = const_pool.tile([P, 8], F32)
            nc.sync.dma_start(out=annr[:], in_=ann_rhs[:, :])
            idn = const_pool.tile([P, P], F32)
            load_chunked(idn, ident, P)

            pa = const_pool.tile([P, 7 * T], F32)
            load_chunked(pa, panch, 7 * T)
            AX1 = pa[:, 0 * T:1 * T]
            AY1 = pa[:, 1 * T:2 * T]
            AX2 = pa[:, 2 * T:3 * T]
            AY2 = pa[:, 3 * T:4 * T]
            AREA = pa[:, 4 * T:5 * T]
            WA = pa[:, 5 * T:6 * T]
            HA = pa[:, 6 * T:7 * T]

            # Each engine observes the DMA semaphores of tensors it will
            # consume, one DMA wait per instruction (a DMA wait plus any
            # other wait exceeds the per-instruction sync-command budget).
            for i, tch in enumerate(touch_list):
                nc.vector.tensor_copy(touch_scr[:, i:i + 1], tch)
            pe_scr = ptr.tile([P, P], F32, tag="pescr")
            nc.tensor.transpose(pe_scr[:], idn[:], idn[:])
            nc.tensor.matmul(pe_scr[:, 0:8], idn[:], annr[:],
                             start=True, stop=True)
            gp_scr = const_pool.tile([P, 1], F32)
            nc.gpsimd.tensor_copy(gp_scr[:], io80[:, 0:1])

            # ---- big state buffers ----
            M = big_pool.tile([P, T], F32)            # max_ov per anchor
            ASG = big_pool.tile([P, T * 8], F32)      # assigned rows (8 cols/tile)
            POS = big_pool.tile([P, T], F32)          # positive mask
            STT = big_pool.tile([P, T], F32)          # states
            REG = big_pool.tile([P, T * 4], F32)      # regression targets (t-major, c-inner)
            SCR = big_pool.tile([P, T], F32)          # scratch

            n_groups = (T + GROUP - 1) // GROUP
            for kg in range(n_groups):
                t0 = kg * GROUP
                t1 = min(T, t0 + GROUP)
                ng = t1 - t0
                asg_ps = pasg.tile([P, GROUP * 8], F32, tag="asg")
                for t in range(t0, t1):
                    ax1 = AX1[:, t:t + 1]
                    ay1 = AY1[:, t:t + 1]
                    ax2 = AX2[:, t:t + 1]
                    ay2 = AY2[:, t:t + 1]
                    area = AREA[:, t:t + 1]

                    maxx = work.tile([P, G], F32, tag="maxx")
                    nc.vector.tensor_scalar(maxx[:], bx1, ax1, None, OP.max)
                    iwr = work.tile([P, G], F32, tag="iwr")
                    nc.vector.scalar_tensor_tensor(
                        iwr[:], bx2, ax2, maxx[:], OP.min, OP.subtract)

                    maxy = work.tile([P, G], F32, tag="maxy")
                    nc.vector.tensor_scalar(maxy[:], by1, ay1, None, OP.max)
                    ihr = work.tile([P, G], F32, tag="ihr")
                    nc.vector.scalar_tensor_tensor(
                        ihr[:], by2, ay2, maxy[:], OP.min, OP.subtract)

                    # iw = relu(iwr) on ACT; inter = relu(ihr) * iw fused on DVE
                    iw = work.tile([P, G], F32, tag="iw")
                    nc.scalar.activation(iw[:], iwr[:], AF.Relu)
                    inter = work.tile([P, G], F32, tag="inter")
                    nc.vector.scalar_tensor_tensor(
                        inter[:], ihr[:], 0.0, iw[:], OP.max, OP.mult)

                    # union = (area_a + area_b) - inter   (exact ref tree)
                    union = work.tile([P, G], F32, tag="union")
                    nc.vector.scalar_tensor_tensor(
                        union[:], areab, area, inter[:], OP.add, OP.subtract)

                    runion = work.tile([P, G], F32, tag="runion")
                    nc.vector.reciprocal(runion[:], union[:])
                    iou = work.tile([P, G], F32, tag="iou")
                    nc.vector.tensor_tensor(iou[:], inter[:], runion[:], OP.mult)

                    # m = max_g iou ; w = (iou == m) * desc ; s = max w ; oh = (w == s)
                    mcol = M[:, t:t + 1]
                    nc.vector.tensor_reduce(mcol, iou[:], axis=AX.X, op=OP.max)
                    w = work.tile([P, G], F32, tag="w")
                    nc.vector.scalar_tensor_tensor(
                        w[:], iou[:], mcol, desc, OP.is_equal, OP.mult)
                    scol = SCR[:, t:t + 1]
                    nc.vector.tensor_reduce(scol, w[:], axis=AX.X, op=OP.max)
                    oh = ohp.tile([P, G], F32, tag="oh")
                    nc.vector.tensor_scalar(oh[:], w[:], scol, None, OP.is_equal)

                    # gather assigned GT row: ohT = oh.T (PE), copy to SBUF, matmul
                    ohT_ps = ptr.tile([P, P], F32, tag="ohT")
                    nc.tensor.transpose(ohT_ps[:], oh[:], idn[:])
                    ohT = ohp.tile([P, P], F32, tag="ohTs")
                    nc.scalar.copy(ohT[:], ohT_ps[:])
                    nc.tensor.matmul(
                        asg_ps[:, (t - t0) * 8:(t - t0) * 8 + 8],
                        ohT[:], annr[:], start=True, stop=True)

                # flush assigned PSUM -> SBUF for this group
                nc.scalar.copy(ASG[:, t0 * 8:t1 * 8], asg_ps[:, :ng * 8])

                # positive mask for the group, then cls one-hot tiles + DMA out
                nc.vector.tensor_scalar(
                    POS[:, t0:t1], M[:, t0:t1], 0.5, None, OP.is_ge)
                for tb in range(t0, t1, CLS_DMA):
                    te = min(t1, tb + CLS_DMA)
                    cls_sb = clsb.tile([P, CLS_DMA * C], F32, tag="cls")
                    for t in range(tb, te):
                        lbl = ASG[:, t * 8 + 4:t * 8 + 5]
                        pcol = POS[:, t:t + 1]
                        nc.gpsimd.tensor_scalar(
                            cls_sb[:, (t - tb) * C:(t - tb + 1) * C],
                            io80[:], lbl, pcol, OP.is_equal, OP.mult)
                    dst = cls_out[:, :].rearrange("(t p) c -> p t c", p=P)
                    nc.sync.dma_start(
                        out=dst[:, tb:te, :],
                        in_=cls_sb[:, :(te - tb) * C].rearrange(
                            "p (t c) -> p t c", c=C))

            # ---- states ----
            nc.vector.tensor_scalar(STT[:], M[:], 0.5, 2.0, OP.is_ge, OP.mult)
            nc.vector.tensor_scalar(SCR[:], M[:], 0.4, -1.0, OP.is_ge, OP.mult)
            nc.vector.tensor_tensor(STT[:], STT[:], SCR[:], OP.add)
            st_dst = st_out[:].rearrange("(t p) -> p t", p=P)
            nch = 8 if T % 8 == 0 else 1
            for i in range(nch):
                sl = slice(i * T // nch, (i + 1) * T // nch)
                nc.sync.dma_start(out=st_dst[:, sl], in_=STT[:, sl])

            # ---- regression targets ----
            # reg_c = ((g_c - a_c) / w_c) / 0.2   with w in {WA,HA,WA,HA}
            RWA = big_pool.tile([P, T], F32)
            RHA = big_pool.tile([P, T], F32)
            nc.vector.reciprocal(RWA[:], WA)
            nc.vector.reciprocal(RHA[:], HA)
            for c, (anch, rw) in enumerate(
                    [(AX1, RWA), (AY1, RHA), (AX2, RWA), (AY2, RHA)]):
                gcol = ASG[:, :].rearrange("p (t e) -> p t e", e=8)[:, :, c]
                rcol = REG[:, :].rearrange("p (t e) -> p t e", e=4)[:, :, c]
                nc.vector.tensor_tensor(rcol, gcol, anch, OP.subtract)
                nc.vector.tensor_tensor(rcol, rcol, rw[:], OP.mult)
            nc.vector.tensor_scalar(REG[:], REG[:], 5.0, None, OP.mult)
            reg_dst = reg_out[:, :].rearrange("(t p) c -> p t c", p=P)
            for i in range(nch):
                sl = slice(i * T // nch * 4, (i + 1) * T // nch * 4)
                slt = slice(i * T // nch, (i + 1) * T // nch)
                nc.sync.dma_start(
                    out=reg_dst[:, slt, :],
                    in_=REG[:, sl].rearrange("p (t c) -> p t c", c=4))

    nc.compile()
    return nc


def host_inputs(annotations_batch, anchors, T=T_FULL):
    """Build per-core input maps (host-side prep, numpy only)."""
    anchors = np.asarray(anchors, dtype=np.float32)
    ann_b = np.asarray(annotations_batch, dtype=np.float32)
    AA = T * P

    ax1 = anchors[:AA, 0].reshape(T, P).T.copy()
    ay1 = anchors[:AA, 1].reshape(T, P).T.copy()
    ax2 = anchors[:AA, 2].reshape(T, P).T.copy()
    ay2 = anchors[:AA, 3].reshape(T, P).T.copy()
    wa = ax2 - ax1
    ha = ay2 - ay1
    area = wa * ha
    panch = np.concatenate([ax1, ay1, ax2, ay2, area, wa, ha], axis=1)

    iota80 = np.broadcast_to(np.arange(C, dtype=np.float32), (P, C)).copy()
    ident = np.eye(P, dtype=np.float32)
    desc_row = (G - np.arange(G)).astype(np.float32)

    in_maps = []
    for b in range(ann_b.shape[0]):
        boxes = ann_b[b, :, :4]
        lbl = ann_b[b, :, 4]
        bx1, by1, bx2, by2 = boxes[:, 0], boxes[:, 1], boxes[:, 2], boxes[:, 3]
        areab = (bx2 - bx1) * (by2 - by1)
        bcast = np.concatenate(
            [np.broadcast_to(r, (P, G)) for r in
             (bx1, by1, bx2, by2, areab, desc_row)], axis=1).astype(np.float32)
        ann_rhs = np.zeros((P, 8), dtype=np.float32)
        ann_rhs[:, 0] = bx1
        ann_rhs[:, 1] = by1
        ann_rhs[:, 2] = bx2
        ann_rhs[:, 3] = by2
        ann_rhs[:, 4] = lbl
        in_maps.append({
            "bcast": np.ascontiguousarray(bcast),
            "iota80": iota80,
            "ann_rhs": ann_rhs,
            "ident": ident,
            "panch": np.ascontiguousarray(panch.astype(np.float32)),
        })
    return in_maps


def kernel(annotations_batch, anchors, T=T_FULL, trace=False):
    key = ("nc", T)
    if key not in _CACHE:
        _CACHE[key] = build_kernel(T)
    nc = _CACHE[key]
    in_maps = host_inputs(annotations_batch, anchors, T)
    n_cores = len(in_maps)
    res = run_bass_kernel_spmd(nc, in_maps, list(range(n_cores)), trace=trace)
    kernel.last_result = res
    cls = np.stack([res.results[i]["cls"] for i in range(n_cores)])
    reg = np.stack([res.results[i]["reg"] for i in range(n_cores)])
    st = np.stack([res.results[i]["st"] for i in range(n_cores)])
    return cls, reg, st
